# revision 28
# baseline (speedup 1.0000x reference)
"""Malvar-He-Cutler demosaic on 8 Trainium2 NeuronCores.

Strategy (W-sharding, bf16, host passthrough/clip), v4 super-tiles:
  - Host reflect-pads x, converts to bf16, column-shards into 8 slices of
    768 cols (+2 halo each side), and splits each shard into column-parity
    planes (halo'd width 772 = even 386 | odd 386).
  - The per-core input is pre-tiled on the HOST into a partition-major
    DRAM layout xp[128, 34*772]: xp[p, j*772+c] = plane row r0(j)+p, where
    r0(j) = 124*j (tile 33 overlaps; rows are duplicated). A super-tile of
    4 row-tiles is then ONE 2D DMA with 6176B contiguous per partition -
    128 descriptors per 4 tiles instead of 128 per tile. Descriptor
    generation (SWDGE Q7) and DMA-engine packet overhead were the
    sustained bottleneck of the per-tile version.
  - Column-symmetric MHC kernels: DVE pre-adds the outer column pairs per
    sub-tile (A = e[c]+e[c+2] at the 2x bf16 DVE mode, B = adjacent-pair
    sums at 1x) so each of the 4 conv maps needs only 3 matmul taps
    (center/A/B): 12 matmul passes per 124-row tile, PSUM-accumulated.
    Banded bf16 stationaries do the vertical 5-tap conv and pack even
    output rows at partitions 0-61, odd at 62-123.
  - 2x2 PSUM tiles x 2 banks = all 8 banks; one ACT activation-copy per
    map pair evicts PSUM f32 -> asm bf16.
  - Output DRAM layout is partition-major too: out[124, 34*1536], so a
    super-tile store is ONE 2D DMA with 12KB contiguous per partition
    (124 descriptors per 4 tiles). Tile 33's 120 duplicate rows are
    stored as-is; the host ignores them at unpack.
  - The passthrough channel (1 of every pixel's 3 channels is x itself)
    and the final clip are done on the HOST: the device only computes
    and ships the 2 interpolated channels per pixel as bf16.
"""

import numpy as np
import ml_dtypes

H, W = 4096, 6144
NCORES = 8
CS = W // NCORES          # 768 cols per core
NC2 = CS // 2             # 384 cols per parity
PHW = NC2 + 2             # 386 parity-plane width (with halo)
XW = 2 * PHW              # 772
TILE_R = 124              # output rows per tile
OUTW = 4 * NC2            # 1536 device-output elems per row (bf16)
NTILES = 34               # 33 full strides + 1 overlap tile
SUP = 4                   # row-tiles per super-tile (DMA batch)

_R0S = [TILE_R * i for i in range(H // TILE_R)] + [H - TILE_R]
_GROUPS = [list(range(g, min(g + SUP, NTILES))) for g in range(0, NTILES, SUP)]

_PROGRAMS = {}


def _build_program(sym=True):
    from concourse import bacc, mybir, tile

    f32 = mybir.dt.float32
    bf16 = mybir.dt.bfloat16
    ntaps = 3 if sym else 5

    nc = bacc.Bacc(None, target_bir_lowering=False, debug=True)
    xp_d = nc.dram_tensor("xp", [128, NTILES * XW], bf16, kind="ExternalInput")
    w_d = nc.dram_tensor("wst", [128, 4 * ntaps * 128], bf16, kind="ExternalInput")
    # 128 rows (not 124): storing all 128 partitions keeps the DMA
    # descriptor swizzle balanced across the 16 SDMA engines; rows 124-127
    # carry zeros (zero stationary columns) and the host ignores them.
    out_d = nc.dram_tensor("out", [128, NTILES * OUTW], bf16, kind="ExternalOutput")

    copy_f = mybir.ActivationFunctionType.Copy

    LOAD_AHEAD = 3   # super-tiles of prefetch
    PSB = 512        # psum bank stride in f32 elements

    with tile.TileContext(nc) as tc:
        with tc.tile_pool(name="wpool", bufs=1) as wpool, \
             tc.tile_pool(name="xpool", bufs=LOAD_AHEAD + 2) as xpool, \
             tc.tile_pool(name="spool", bufs=LOAD_AHEAD + 2) as spool, \
             tc.tile_pool(name="apool", bufs=3) as apool, \
             tc.tile_pool(name="ppool", bufs=2, space="PSUM") as ppool:

            wt = wpool.tile([128, 4 * ntaps * 128], bf16, name="wt")
            nc.sync.dma_start(out=wt[:], in_=w_d.ap())



            def issue_load(grp, split_first=False):
                n = len(grp)
                Xt = xpool.tile([128, SUP * XW], bf16, name="X", tag="X")
                if split_first:
                    # 1-tile first load so the first sub-tile's compute can
                    # start as soon as it lands (shorter ramp)
                    nc.gpsimd.dma_start(
                        out=Xt[:, 0:XW],
                        in_=xp_d[:, grp[0] * XW : (grp[0] + 1) * XW],
                    )
                    nc.gpsimd.dma_start(
                        out=Xt[:, XW : n * XW],
                        in_=xp_d[:, (grp[0] + 1) * XW : (grp[0] + n) * XW],
                    )
                else:
                    nc.gpsimd.dma_start(
                        out=Xt[:, 0 : n * XW],
                        in_=xp_d[:, grp[0] * XW : (grp[0] + n) * XW],
                    )
                if not sym:
                    return Xt, None
                # horizontal pair pre-sums per sub-tile, issued with the load
                # so DVE runs ahead of the PE. S sub-layout = [A_e|B_e|B_o|A_o]
                #   A_e = e[c]+e[c+2]   (2x DVE mode: both operands 4B-aligned)
                #   B_e = o[c]+o[c+1]   (1x: +1 shift misaligns one operand)
                #   B_o = e[c+1]+e[c+2] (1x)
                #   A_o = o[c]+o[c+2]   (2x)
                S = spool.tile([128, SUP * 4 * NC2], bf16, name="S", tag="S")
                ta = nc.vector.tensor_add
                for k in range(n):
                    xb = k * XW
                    sb = k * 4 * NC2
                    ta(S[:, sb : sb + NC2],
                       Xt[:, xb : xb + NC2], Xt[:, xb + 2 : xb + NC2 + 2])
                    ta(S[:, sb + NC2 : sb + 2 * NC2],
                       Xt[:, xb + PHW : xb + PHW + NC2],
                       Xt[:, xb + PHW + 1 : xb + PHW + NC2 + 1])
                    ta(S[:, sb + 2 * NC2 : sb + 3 * NC2],
                       Xt[:, xb + 1 : xb + NC2 + 1], Xt[:, xb + 2 : xb + NC2 + 2])
                    ta(S[:, sb + 3 * NC2 : sb + 4 * NC2],
                       Xt[:, xb + PHW : xb + PHW + NC2],
                       Xt[:, xb + PHW + 2 : xb + PHW + NC2 + 2])
                return Xt, S

            def store(t0, k0, n, asm):
                # tiles [t0, t0+n) from asm sub-slots [k0, k0+n)
                nc.gpsimd.dma_start(
                    out=out_d[:, t0 * OUTW : (t0 + n) * OUTW],
                    in_=asm[0:128, k0 * OUTW : (k0 + n) * OUTW],
                )

            loaded = {g: issue_load(_GROUPS[g], split_first=(g == 0))
                      for g in range(min(LOAD_AHEAD + 1, len(_GROUPS)))}
            for gi, grp in enumerate(_GROUPS):
                X, S = loaded.pop(gi)
                if gi + LOAD_AHEAD + 1 < len(_GROUPS):
                    gg = gi + LOAD_AHEAD + 1
                    loaded[gg] = issue_load(_GROUPS[gg])

                asm = apool.tile([128, SUP * OUTW], bf16, name="asm", tag="asm")
                for k in range(len(grp)):
                    xb = k * XW
                    sb = k * 4 * NC2
                    if sym:
                        mov_e = [X[:, xb + 1 : xb + NC2 + 1],
                                 S[:, sb : sb + NC2],
                                 S[:, sb + NC2 : sb + 2 * NC2]]
                        mov_o = [X[:, xb + PHW + 1 : xb + PHW + NC2 + 1],
                                 S[:, sb + 3 * NC2 : sb + 4 * NC2],
                                 S[:, sb + 2 * NC2 : sb + 3 * NC2]]
                    else:
                        mov_e = [X[:, xb : xb + NC2],
                                 X[:, xb + 1 : xb + NC2 + 1],
                                 X[:, xb + 2 : xb + NC2 + 2],
                                 X[:, xb + PHW : xb + PHW + NC2],
                                 X[:, xb + PHW + 1 : xb + PHW + NC2 + 1]]
                        mov_o = [X[:, xb + PHW : xb + PHW + NC2],
                                 X[:, xb + PHW + 1 : xb + PHW + NC2 + 1],
                                 X[:, xb + PHW + 2 : xb + PHW + NC2 + 2],
                                 X[:, xb + 1 : xb + NC2 + 1],
                                 X[:, xb + 2 : xb + NC2 + 2]]
                    movs = [mov_e, mov_e, mov_o, mov_o]  # E1, E2, O1, O2

                    pstiles = [
                        ppool.tile([128, 2 * PSB], f32, name=f"ps{p}", tag=f"ps{p}")
                        for p in range(2)
                    ]

                    for m in range(4):
                        for s_i in range(ntaps):
                            ps = pstiles[m // 2]
                            nc.tensor.matmul(
                                ps[:, (m % 2) * PSB : (m % 2) * PSB + NC2],
                                lhsT=wt[:, (m * ntaps + s_i) * 128
                                        : (m * ntaps + s_i + 1) * 128],
                                rhs=movs[m][s_i],
                                start=(s_i == 0),
                                stop=(s_i == ntaps - 1),
                            )
                        if m % 2 == 1:
                            pair = m // 2
                            src = pstiles[pair][0:128, :].rearrange(
                                "p (b f) -> p b f", b=2
                            )[:, :, 0:NC2]
                            dst = asm[0:128,
                                      sb + 2 * pair * NC2
                                      : sb + 2 * (pair + 1) * NC2]
                            nc.scalar.activation(dst, src, copy_f)

                    # store each 2-tile half as soon as it is evicted, so
                    # store DMA is spread through compute instead of
                    # piling into an end-of-kernel drain; the last group
                    # stores per tile so the final drain is minimal
                    if gi == len(_GROUPS) - 1:
                        store(grp[k], k, 1, asm)
                    elif k % 2 == 1:
                        store(grp[k - 1], k - 1, 2, asm)


    nc.compile()
    return nc


def _get_program(sym):
    if sym not in _PROGRAMS:
        _PROGRAMS[sym] = _build_program(sym)
    return _PROGRAMS[sym]


def _build_stationary(kern, sym):
    """kern: [4,5,5] f32 -> W [128, 4*ntaps*128] bf16 (lhsT per tap)."""
    groups = [(0, 2), (3, 1), (1, 3), (2, 0)]  # (even-row kernel, odd-row kernel)
    ntaps = 3 if sym else 5
    Wm = np.zeros((4 * ntaps, 128, 128), np.float32)
    t = np.arange(62)
    for m, (ka, kb) in enumerate(groups):
        if sym:
            profs_a = [kern[ka][:, 2], kern[ka][:, 0], kern[ka][:, 1]]
            profs_b = [kern[kb][:, 2], kern[kb][:, 0], kern[kb][:, 1]]
        else:
            # stream order must match mov_e/mov_o: [p0, p1(center), p2, q0, q1]
            cols = [0, 2, 4, 1, 3]
            profs_a = [kern[ka][:, c] for c in cols]
            profs_b = [kern[kb][:, c] for c in cols]
        for s in range(ntaps):
            Wq = Wm[m * ntaps + s]
            for di in range(5):
                Wq[2 * t + di, t] += profs_a[s][di]        # even out rows -> p 0-61
                Wq[2 * t + 1 + di, 62 + t] += profs_b[s][di]  # odd out rows -> p 62-123
    out = np.ascontiguousarray(Wm.transpose(1, 0, 2).reshape(128, 4 * ntaps * 128))
    return out.astype(ml_dtypes.bfloat16)


def kernel(x, kernels, _trace=False):
    from concourse.bass_utils import run_bass_kernel_spmd

    x = np.asarray(x, dtype=np.float32)
    kern = np.asarray(kernels, dtype=np.float32).reshape(4, 5, 5)
    sym = bool(
        np.array_equal(kern[:, :, 0], kern[:, :, 4])
        and np.array_equal(kern[:, :, 1], kern[:, :, 3])
    )
    wst = _build_stationary(kern, sym)
    xpad = np.pad(x, 2, mode="reflect").astype(ml_dtypes.bfloat16)

    r0s = np.asarray(_R0S)
    row_idx = r0s[:, None] + np.arange(128)[None, :]  # [34, 128]

    in_maps = []
    for c in range(NCORES):
        sh = xpad[:, c * CS : c * CS + CS + 4]  # [4100, 772]
        shp = np.concatenate([sh[:, 0::2], sh[:, 1::2]], axis=1)
        # partition-major pre-tiling: xp[p, j, :] = shp[r0(j)+p, :]
        xp = np.ascontiguousarray(
            shp[row_idx].transpose(1, 0, 2).reshape(128, NTILES * XW)
        )
        in_maps.append({"xp": xp, "wst": wst})

    nc = _get_program(sym)
    res = run_bass_kernel_spmd(nc, in_maps, list(range(NCORES)), trace=_trace)

    # device out: [124, 34, 4, NC2]; partition p = h*62+i -> image row
    # r0(j) + 2i + h. Unpack into the full half-res col grid.
    secs = np.empty((H, 4, NCORES, NC2), np.float32)
    for c in range(NCORES):
        dev = np.asarray(res.results[c]["out"]).reshape(128, NTILES, 4, NC2)[:TILE_R]
        for j, r0 in enumerate(_R0S):
            blk = dev[:, j].reshape(2, 62, 4, NC2)
            secs[r0 : r0 + TILE_R : 2, :, c, :] = blk[0]
            secs[r0 + 1 : r0 + TILE_R : 2, :, c, :] = blk[1]
    E1 = np.clip(secs[:, 0].reshape(H, W // 2), 0.0, 1.0)
    E2 = np.clip(secs[:, 1].reshape(H, W // 2), 0.0, 1.0)
    O1 = np.clip(secs[:, 2].reshape(H, W // 2), 0.0, 1.0)
    O2 = np.clip(secs[:, 3].reshape(H, W // 2), 0.0, 1.0)
    xc = np.clip(x, 0.0, 1.0)

    out = np.empty((H, W, 3), np.float32)
    # R channel
    out[0::2, 0::2, 0] = xc[0::2, 0::2]
    out[1::2, 0::2, 0] = E1[1::2]
    out[:, 1::2, 0] = O1
    # G channel
    out[0::2, 0::2, 1] = E1[0::2]
    out[1::2, 0::2, 1] = xc[1::2, 0::2]
    out[0::2, 1::2, 1] = xc[0::2, 1::2]
    out[1::2, 1::2, 1] = O2[1::2]
    # B channel
    out[:, 0::2, 2] = E2
    out[0::2, 1::2, 2] = O2[0::2]
    out[1::2, 1::2, 2] = xc[1::2, 1::2]

    if _trace:
        return out, res
    return out


# revision 31
# speedup vs baseline: 1.0226x; 1.0226x over previous
"""Malvar-He-Cutler demosaic on 8 Trainium2 NeuronCores.

Strategy (W-sharding, bf16, host passthrough/clip), v4 super-tiles:
  - Host reflect-pads x, converts to bf16, column-shards into 8 slices of
    768 cols (+2 halo each side), and splits each shard into column-parity
    planes (halo'd width 772 = even 386 | odd 386).
  - The per-core input is pre-tiled on the HOST into a partition-major
    DRAM layout xp[128, 34*772]: xp[p, j*772+c] = plane row r0(j)+p, where
    r0(j) = 124*j (tile 33 overlaps; rows are duplicated). A super-tile of
    4 row-tiles is then ONE 2D DMA with 6176B contiguous per partition -
    128 descriptors per 4 tiles instead of 128 per tile. Descriptor
    generation (SWDGE Q7) and DMA-engine packet overhead were the
    sustained bottleneck of the per-tile version.
  - Column-symmetric MHC kernels: DVE pre-adds the outer column pairs per
    sub-tile (A = e[c]+e[c+2] at the 2x bf16 DVE mode, B = adjacent-pair
    sums at 1x) so each of the 4 conv maps needs only 3 matmul taps
    (center/A/B): 12 matmul passes per 124-row tile, PSUM-accumulated.
    Banded bf16 stationaries do the vertical 5-tap conv and pack even
    output rows at partitions 0-61, odd at 62-123.
  - 2x2 PSUM tiles x 2 banks = all 8 banks; one ACT activation-copy per
    map pair evicts PSUM f32 -> asm bf16.
  - Output DRAM layout is partition-major too: out[128, 34*1536]. All
    128 partitions are stored (rows 124-127 are zeros from the zero
    stationary columns) so the DMA descriptor swizzle stays balanced
    across the 16 SDMA engines. Each 2-tile half of a super-tile is
    stored as ONE 2D DMA (6KB contiguous per partition) as soon as its
    evictions land, spreading store traffic through compute; the last
    group stores per tile so the final drain is minimal. Tile 33's 120
    duplicate rows are stored as-is; the host ignores them at unpack.
  - The passthrough channel (1 of every pixel's 3 channels is x itself)
    and the final clip are done on the HOST: the device only computes
    and ships the 2 interpolated channels per pixel as bf16.
"""

import numpy as np
import ml_dtypes

H, W = 4096, 6144
NCORES = 8
CS = W // NCORES          # 768 cols per core
NC2 = CS // 2             # 384 cols per parity
PHW = NC2 + 2             # 386 parity-plane width (with halo)
XW = 2 * PHW              # 772
TILE_R = 124              # output rows per tile
OUTW = 4 * NC2            # 1536 device-output elems per row (bf16)
NTILES = 34               # 33 full strides + 1 overlap tile
SUP = 4                   # row-tiles per super-tile (DMA batch)

_R0S = [TILE_R * i for i in range(H // TILE_R)] + [H - TILE_R]
_GROUPS = [list(range(g, min(g + SUP, NTILES))) for g in range(0, NTILES, SUP)]

_PROGRAMS = {}


def _build_program(sym=True):
    from concourse import bacc, mybir, tile

    f32 = mybir.dt.float32
    bf16 = mybir.dt.bfloat16
    ntaps = 3 if sym else 5

    nc = bacc.Bacc(None, target_bir_lowering=False, debug=True)
    xp_d = nc.dram_tensor("xp", [128, NTILES * XW], bf16, kind="ExternalInput")
    w_d = nc.dram_tensor("wst", [128, 4 * ntaps * 128], bf16, kind="ExternalInput")
    # 128 rows (not 124): storing all 128 partitions keeps the DMA
    # descriptor swizzle balanced across the 16 SDMA engines; rows 124-127
    # carry zeros (zero stationary columns) and the host ignores them.
    out_d = nc.dram_tensor("out", [128, NTILES * OUTW], bf16, kind="ExternalOutput")

    copy_f = mybir.ActivationFunctionType.Copy

    LOAD_AHEAD = 3   # super-tiles of prefetch
    PSB = 512        # psum bank stride in f32 elements

    with tile.TileContext(nc) as tc:
        with tc.tile_pool(name="wpool", bufs=1) as wpool, \
             tc.tile_pool(name="xpool", bufs=LOAD_AHEAD + 2) as xpool, \
             tc.tile_pool(name="spool", bufs=LOAD_AHEAD + 2) as spool, \
             tc.tile_pool(name="apool", bufs=3) as apool, \
             tc.tile_pool(name="ppool", bufs=2, space="PSUM") as ppool:

            wt = wpool.tile([128, 4 * ntaps * 128], bf16, name="wt")
            nc.sync.dma_start(out=wt[:], in_=w_d.ap())



            def issue_load(grp, split_first=False):
                n = len(grp)
                Xt = xpool.tile([128, SUP * XW], bf16, name="X", tag="X")
                if split_first:
                    # 1-tile first load so the first sub-tile's compute can
                    # start as soon as it lands (shorter ramp)
                    nc.gpsimd.dma_start(
                        out=Xt[:, 0:XW],
                        in_=xp_d[:, grp[0] * XW : (grp[0] + 1) * XW],
                    )
                    nc.gpsimd.dma_start(
                        out=Xt[:, XW : n * XW],
                        in_=xp_d[:, (grp[0] + 1) * XW : (grp[0] + n) * XW],
                    )
                else:
                    nc.gpsimd.dma_start(
                        out=Xt[:, 0 : n * XW],
                        in_=xp_d[:, grp[0] * XW : (grp[0] + n) * XW],
                    )
                if not sym:
                    return Xt, None
                # horizontal pair pre-sums per sub-tile, issued with the load
                # so DVE runs ahead of the PE. S sub-layout = [A_e|B_e|B_o|A_o]
                #   A_e = e[c]+e[c+2]   (2x DVE mode: both operands 4B-aligned)
                #   B_e = o[c]+o[c+1]   (1x: +1 shift misaligns one operand)
                #   B_o = e[c+1]+e[c+2] (1x)
                #   A_o = o[c]+o[c+2]   (2x)
                S = spool.tile([128, SUP * 4 * NC2], bf16, name="S", tag="S")
                ta = nc.vector.tensor_add
                for k in range(n):
                    xb = k * XW
                    sb = k * 4 * NC2
                    ta(S[:, sb : sb + NC2],
                       Xt[:, xb : xb + NC2], Xt[:, xb + 2 : xb + NC2 + 2])
                    ta(S[:, sb + NC2 : sb + 2 * NC2],
                       Xt[:, xb + PHW : xb + PHW + NC2],
                       Xt[:, xb + PHW + 1 : xb + PHW + NC2 + 1])
                    ta(S[:, sb + 2 * NC2 : sb + 3 * NC2],
                       Xt[:, xb + 1 : xb + NC2 + 1], Xt[:, xb + 2 : xb + NC2 + 2])
                    ta(S[:, sb + 3 * NC2 : sb + 4 * NC2],
                       Xt[:, xb + PHW : xb + PHW + NC2],
                       Xt[:, xb + PHW + 2 : xb + PHW + NC2 + 2])
                return Xt, S

            def store(t0, k0, n, asm):
                # tiles [t0, t0+n) from asm sub-slots [k0, k0+n)
                nc.gpsimd.dma_start(
                    out=out_d[:, t0 * OUTW : (t0 + n) * OUTW],
                    in_=asm[0:128, k0 * OUTW : (k0 + n) * OUTW],
                )

            loaded = {g: issue_load(_GROUPS[g], split_first=(g == 0))
                      for g in range(min(LOAD_AHEAD + 1, len(_GROUPS)))}
            for gi, grp in enumerate(_GROUPS):
                X, S = loaded.pop(gi)
                if gi + LOAD_AHEAD + 1 < len(_GROUPS):
                    gg = gi + LOAD_AHEAD + 1
                    loaded[gg] = issue_load(_GROUPS[gg])

                asm = apool.tile([128, SUP * OUTW], bf16, name="asm", tag="asm")
                for k in range(len(grp)):
                    xb = k * XW
                    sb = k * 4 * NC2
                    if sym:
                        mov_e = [X[:, xb + 1 : xb + NC2 + 1],
                                 S[:, sb : sb + NC2],
                                 S[:, sb + NC2 : sb + 2 * NC2]]
                        mov_o = [X[:, xb + PHW + 1 : xb + PHW + NC2 + 1],
                                 S[:, sb + 3 * NC2 : sb + 4 * NC2],
                                 S[:, sb + 2 * NC2 : sb + 3 * NC2]]
                    else:
                        mov_e = [X[:, xb : xb + NC2],
                                 X[:, xb + 1 : xb + NC2 + 1],
                                 X[:, xb + 2 : xb + NC2 + 2],
                                 X[:, xb + PHW : xb + PHW + NC2],
                                 X[:, xb + PHW + 1 : xb + PHW + NC2 + 1]]
                        mov_o = [X[:, xb + PHW : xb + PHW + NC2],
                                 X[:, xb + PHW + 1 : xb + PHW + NC2 + 1],
                                 X[:, xb + PHW + 2 : xb + PHW + NC2 + 2],
                                 X[:, xb + 1 : xb + NC2 + 1],
                                 X[:, xb + 2 : xb + NC2 + 2]]
                    movs = [mov_e, mov_e, mov_o, mov_o]  # E1, E2, O1, O2

                    pstiles = [
                        ppool.tile([128, 2 * PSB], f32, name=f"ps{p}", tag=f"ps{p}")
                        for p in range(2)
                    ]

                    for m in range(4):
                        for s_i in range(ntaps):
                            ps = pstiles[m // 2]
                            nc.tensor.matmul(
                                ps[:, (m % 2) * PSB : (m % 2) * PSB + NC2],
                                lhsT=wt[:, (m * ntaps + s_i) * 128
                                        : (m * ntaps + s_i + 1) * 128],
                                rhs=movs[m][s_i],
                                start=(s_i == 0),
                                stop=(s_i == ntaps - 1),
                            )
                        if m % 2 == 1:
                            pair = m // 2
                            src = pstiles[pair][0:128, :].rearrange(
                                "p (b f) -> p b f", b=2
                            )[:, :, 0:NC2]
                            dst = asm[0:128,
                                      sb + 2 * pair * NC2
                                      : sb + 2 * (pair + 1) * NC2]
                            nc.scalar.activation(dst, src, copy_f)

                    # store each 2-tile half as soon as it is evicted, so
                    # store DMA is spread through compute instead of
                    # piling into an end-of-kernel drain; the last group
                    # stores per tile so the final drain is minimal
                    if gi == len(_GROUPS) - 1:
                        store(grp[k], k, 1, asm)
                    elif k % 2 == 1:
                        store(grp[k - 1], k - 1, 2, asm)


    nc.compile()
    return nc


def _get_program(sym):
    if sym not in _PROGRAMS:
        _PROGRAMS[sym] = _build_program(sym)
    return _PROGRAMS[sym]


def _build_stationary(kern, sym):
    """kern: [4,5,5] f32 -> W [128, 4*ntaps*128] bf16 (lhsT per tap)."""
    groups = [(0, 2), (3, 1), (1, 3), (2, 0)]  # (even-row kernel, odd-row kernel)
    ntaps = 3 if sym else 5
    Wm = np.zeros((4 * ntaps, 128, 128), np.float32)
    t = np.arange(62)
    for m, (ka, kb) in enumerate(groups):
        if sym:
            profs_a = [kern[ka][:, 2], kern[ka][:, 0], kern[ka][:, 1]]
            profs_b = [kern[kb][:, 2], kern[kb][:, 0], kern[kb][:, 1]]
        else:
            # stream order must match mov_e/mov_o: [p0, p1(center), p2, q0, q1]
            cols = [0, 2, 4, 1, 3]
            profs_a = [kern[ka][:, c] for c in cols]
            profs_b = [kern[kb][:, c] for c in cols]
        for s in range(ntaps):
            Wq = Wm[m * ntaps + s]
            for di in range(5):
                Wq[2 * t + di, t] += profs_a[s][di]        # even out rows -> p 0-61
                Wq[2 * t + 1 + di, 62 + t] += profs_b[s][di]  # odd out rows -> p 62-123
    out = np.ascontiguousarray(Wm.transpose(1, 0, 2).reshape(128, 4 * ntaps * 128))
    return out.astype(ml_dtypes.bfloat16)


def kernel(x, kernels, _trace=False):
    from concourse.bass_utils import run_bass_kernel_spmd

    x = np.asarray(x, dtype=np.float32)
    kern = np.asarray(kernels, dtype=np.float32).reshape(4, 5, 5)
    sym = bool(
        np.array_equal(kern[:, :, 0], kern[:, :, 4])
        and np.array_equal(kern[:, :, 1], kern[:, :, 3])
    )
    wst = _build_stationary(kern, sym)
    xpad = np.pad(x, 2, mode="reflect").astype(ml_dtypes.bfloat16)

    r0s = np.asarray(_R0S)
    row_idx = r0s[:, None] + np.arange(128)[None, :]  # [34, 128]

    in_maps = []
    for c in range(NCORES):
        sh = xpad[:, c * CS : c * CS + CS + 4]  # [4100, 772]
        shp = np.concatenate([sh[:, 0::2], sh[:, 1::2]], axis=1)
        # partition-major pre-tiling: xp[p, j, :] = shp[r0(j)+p, :]
        xp = np.ascontiguousarray(
            shp[row_idx].transpose(1, 0, 2).reshape(128, NTILES * XW)
        )
        in_maps.append({"xp": xp, "wst": wst})

    nc = _get_program(sym)
    res = run_bass_kernel_spmd(nc, in_maps, list(range(NCORES)), trace=_trace)

    # device out: [124, 34, 4, NC2]; partition p = h*62+i -> image row
    # r0(j) + 2i + h. Unpack into the full half-res col grid.
    secs = np.empty((H, 4, NCORES, NC2), np.float32)
    for c in range(NCORES):
        dev = np.asarray(res.results[c]["out"]).reshape(128, NTILES, 4, NC2)[:TILE_R]
        for j, r0 in enumerate(_R0S):
            blk = dev[:, j].reshape(2, 62, 4, NC2)
            secs[r0 : r0 + TILE_R : 2, :, c, :] = blk[0]
            secs[r0 + 1 : r0 + TILE_R : 2, :, c, :] = blk[1]
    E1 = np.clip(secs[:, 0].reshape(H, W // 2), 0.0, 1.0)
    E2 = np.clip(secs[:, 1].reshape(H, W // 2), 0.0, 1.0)
    O1 = np.clip(secs[:, 2].reshape(H, W // 2), 0.0, 1.0)
    O2 = np.clip(secs[:, 3].reshape(H, W // 2), 0.0, 1.0)
    xc = np.clip(x, 0.0, 1.0)

    out = np.empty((H, W, 3), np.float32)
    # R channel
    out[0::2, 0::2, 0] = xc[0::2, 0::2]
    out[1::2, 0::2, 0] = E1[1::2]
    out[:, 1::2, 0] = O1
    # G channel
    out[0::2, 0::2, 1] = E1[0::2]
    out[1::2, 0::2, 1] = xc[1::2, 0::2]
    out[0::2, 1::2, 1] = xc[0::2, 1::2]
    out[1::2, 1::2, 1] = O2[1::2]
    # B channel
    out[:, 0::2, 2] = E2
    out[0::2, 1::2, 2] = O2[0::2]
    out[1::2, 1::2, 2] = xc[1::2, 1::2]

    if _trace:
        return out, res
    return out
